# revision 21
# baseline (speedup 1.0000x reference)
"""MoE top-2 routing kernel for Trainium2, 8-core data-parallel, int8 wire.

Problem: x [524288, 128] f32; gate Linear(128->8); 8 experts Linear(128->128).
  g = softmax(x @ gate_W.T + gate_b); top-2 mask; out = sum_e (g*mask)_e * (x @ W_e.T) + g @ b

The axon tunnel to the 8 NeuronCores moves ~40-48 MB/s half-duplex, so wall
time is dominated by wire bytes.  Strategy:
  host (jax CPU jit): gating/softmax/top-2 in f32 (exact), per-token int8
    quantization of x, dequant scale folded into the gate weights; final
    dequant of the device's int8 output.
  wire in:  one blob per core = xq int8 + gm*scale fp16 + gT fp16 (10MB/core)
  device:   int8->f32r convert, PE transpose, f32r expert matmuls, weighted
    bf16 reduce, g@b bias matmul, per-token amax + int8 quantization (HW
    converter rounds to nearest).
  wire out: one blob per core = yq int8 + r127 scale f32 (132B/token)

The PJRT exec path mirrors concourse.bass2jax.run_bass_via_pjrt but caches
the jitted sharded callable and keeps constant weights device-resident.
Host prep / transfers / fetch / dequant are pipelined per core.
"""

import sys

if "/opt/trn_rl_repo" not in sys.path:
    sys.path.insert(0, "/opt/trn_rl_repo")

from contextlib import ExitStack

import numpy as np

import concourse.bass as bass
import concourse.tile as tile
from concourse import bacc
from concourse import mybir

F32 = mybir.dt.float32
F32R = mybir.dt.float32r
BF16 = mybir.dt.bfloat16
F16 = mybir.dt.float16
I8 = mybir.dt.int8
U8 = mybir.dt.uint8
AF = mybir.ActivationFunctionType
OP = mybir.AluOpType
AX = mybir.AxisListType

N_TOKENS = 524288
D = 128
E = 8
N_CORES = 8
P = 128
G = 16  # tiles per group
SHARD = N_TOKENS // N_CORES  # 65536 tokens per core
NGROUP = SHARD // (P * G)  # 32 groups per core

# blob row layout (128-byte rows, uint8)
ROWS_XQ = SHARD  # int8 x, one row per token
ROWS_GMP = SHARD * E * 2 // 128  # fp16 gm*scale, packed per group
ROWS_GT = SHARD * E // 128  # uint8 g*255 transposed [E, SHARD]
ROWS_IN = ROWS_XQ + ROWS_GMP + ROWS_GT
OUT_ROW_BYTES = D + 2  # int8 y row + fp16 r127 scale


def _bcast_inner(ap, n_outer, rep_len):
    """View [P, n_outer] as [P, n_outer, rep_len] with inner dim broadcast."""
    return bass.AP(
        tensor=ap.tensor,
        offset=ap.offset,
        ap=[ap.ap[0], [ap.ap[-1][0], n_outer], [0, rep_len]],
    )


def build_nc(shard_tokens: int = SHARD, inner_tiles: int = G) -> bass.Bass:
    ntiles = shard_tokens // P
    assert ntiles % inner_tiles == 0
    outer = ntiles // inner_tiles
    gi = inner_tiles
    rows_xq = shard_tokens
    rows_gmp = shard_tokens * E * 2 // 128
    rows_gt = shard_tokens * E // 128

    nc = bacc.Bacc()
    blob = nc.dram_tensor(
        "blob", [rows_xq + rows_gmp + rows_gt, 128], U8, kind="ExternalInput"
    )
    # wcat[d, e*128+f] = W[e, f, d]
    wcat = nc.dram_tensor("wcat", [D, E * D], F32R, kind="ExternalInput")
    bmat = nc.dram_tensor("bmat", [E, D], F16, kind="ExternalInput")
    ident_f = nc.dram_tensor("ident_f", [P, P], F32R, kind="ExternalInput")
    obuf = nc.dram_tensor(
        "obuf", [shard_tokens, OUT_ROW_BYTES], U8, kind="ExternalOutput"
    )

    # views into the blobs
    x_v = (
        blob[0:rows_xq, :]
        .bitcast(I8)
        .rearrange("(n a p) d -> n p a d", p=P, a=gi)
    )
    # gmp logical [outer*P, gi*E] fp16: gmp[group*P + p, j*E + e]
    gm_v = (
        blob[rows_xq : rows_xq + rows_gmp, :]
        .rearrange("(r two) c -> r (two c)", two=2)
        .bitcast(F16)
        .rearrange("(n p) ge -> n p ge", p=P)
    )
    # gt logical [E, shard] uint8 (g * 255)
    gt_v = (
        blob[rows_xq + rows_gmp :, :]
        .rearrange("(e r) c -> e (r c)", e=E)
        .rearrange("e (n t) -> n e t", t=P * gi)
    )
    yq_v = obuf[:, 0:D].bitcast(I8).rearrange("(n a p) d -> n p a d", p=P, a=gi)
    sy_v = (
        obuf[:, D : D + 2]
        .bitcast(F16)
        .rearrange("(n a p) one -> n p (a one)", p=P, a=gi)
    )

    with ExitStack() as ctx:
        tc = ctx.enter_context(tile.TileContext(nc))
        consts = ctx.enter_context(tc.tile_pool(name="consts", bufs=1))
        io_pool = ctx.enter_context(tc.tile_pool(name="io", bufs=2))
        xt_pool = ctx.enter_context(tc.tile_pool(name="xts", bufs=2))
        work = ctx.enter_context(tc.tile_pool(name="work", bufs=2))
        gates = ctx.enter_context(tc.tile_pool(name="gates", bufs=2))
        psum_y = ctx.enter_context(tc.tile_pool(name="psum_y", bufs=2, space="PSUM"))
        psum_t = ctx.enter_context(tc.tile_pool(name="psum_t", bufs=2, space="PSUM"))

        # ---- constants (one-time) ----
        wcat_sb = consts.tile([D, E * D], F32R)
        nc.sync.dma_start(out=wcat_sb, in_=wcat[:, :])
        bmat_sb = consts.tile([E, D], F16)
        nc.sync.dma_start(out=bmat_sb, in_=bmat[:, :])
        ident_r = consts.tile([P, P], F32R)
        nc.sync.dma_start(out=ident_r, in_=ident_f[:, :])

        def body(base):
            xq_in = io_pool.tile([P, gi, D], I8, tag="xq_in")
            nc.sync.dma_start(out=xq_in, in_=x_v[base])
            gm_in = gates.tile([P, gi * E], F16, tag="gm_in")
            nc.sync.dma_start(out=gm_in, in_=gm_v[base])
            gmf = gates.tile([P, gi * E], F32, tag="gmf")
            nc.vector.tensor_copy(out=gmf, in_=gm_in)
            gt_u8 = gates.tile([E, gi * P], U8, tag="gt_u8")
            nc.sync.dma_start(out=gt_u8, in_=gt_v[base])
            gt_sb = gates.tile([E, gi * P], F16, tag="gt_sb")
            nc.vector.tensor_copy(out=gt_sb, in_=gt_u8)

            xts = xt_pool.tile([P, gi, D], F32R, tag="xts")
            yq_sb = io_pool.tile([P, gi, D], I8, tag="yq_sb")
            sy_sb = io_pool.tile([P, gi], F16, tag="sy_sb")

            for j in range(gi):
                # int8 -> f32r (exact, ACT convert), then PE transpose -> xT
                xf = work.tile([P, D], F32R, tag="xf")
                nc.scalar.activation(xf, xq_in[:, j, :], AF.Copy)
                tp = psum_t.tile([P, D], F32, tag="tp")
                nc.tensor.transpose(tp.bitcast(F32R), xf, ident_r)
                nc.scalar.copy(xts[:, j, :], tp)

                yp = psum_y.tile([P, E * D], F32, tag="yall")
                nc.tensor.matmul(
                    yp[:, 0:512], xts[:, j, :], wcat_sb[:, 0:512], start=True, stop=True
                )
                nc.tensor.matmul(
                    yp[:, 512:1024],
                    xts[:, j, :],
                    wcat_sb[:, 512:1024],
                    start=True,
                    stop=True,
                )

                # weighted reduce: sc[p, e, f] = yall[p, e, f] * gm_scaled[p, j, e]
                sc = work.tile([P, E, D], BF16, tag="sc")
                yp3 = yp.rearrange("p (e f) -> p e f", f=D)
                ghj = gmf[:, j * E : (j + 1) * E]
                ghb = bass.AP(
                    tensor=ghj.tensor,
                    offset=ghj.offset,
                    ap=[ghj.ap[0], [1, 6], [0, D]],
                )
                nc.vector.tensor_tensor(
                    out=sc[:, 0:6, :], in0=yp3[:, 0:6, :], in1=ghb, op=OP.mult
                )
                for e in (6, 7):
                    nc.scalar.activation(
                        sc[:, e, :], yp3[:, e, :], AF.Copy, scale=ghj[:, e : e + 1]
                    )
                # bf16 add tree: level 1 on gpsimd, 2 on DVE, final f32 on DVE
                sc4 = work.tile([P, 4, D], BF16, tag="sc4")
                nc.gpsimd.tensor_tensor(
                    out=sc4, in0=sc[:, 0:4, :], in1=sc[:, 4:8, :], op=OP.add
                )
                sc2 = work.tile([P, 2, D], BF16, tag="sc2")
                nc.vector.tensor_tensor(
                    out=sc2, in0=sc4[:, 0:2, :], in1=sc4[:, 2:4, :], op=OP.add
                )
                # bias term: bp[p, f] = sum_e g[p, e] * b[e, f]
                bp = psum_t.tile([P, D], F32, tag="bp")
                nc.tensor.matmul(
                    bp,
                    gt_sb[:, j * P : (j + 1) * P],
                    bmat_sb,
                    start=True,
                    stop=True,
                )
                s0 = work.tile([P, D], F32, tag="s0")
                nc.vector.tensor_tensor(
                    out=s0, in0=sc2[:, 0, :], in1=sc2[:, 1, :], op=OP.add
                )
                s1f = work.tile([P, D], F32, tag="s1f")
                nc.vector.tensor_tensor(out=s1f, in0=s0, in1=bp, op=OP.add)

                # per-token int8 quantization: q = rne(y * 127/amax)
                ab = work.tile([P, D], F32, tag="ab")
                nc.scalar.activation(ab, s1f, AF.Abs)
                mx = work.tile([P, 1], F32, tag="mx")
                nc.vector.tensor_reduce(out=mx, in_=ab, axis=AX.X, op=OP.max)
                nc.vector.tensor_scalar(
                    out=mx, in0=mx, scalar1=1e-30, scalar2=None, op0=OP.max
                )
                rv = work.tile([P, 1], F32, tag="rv")
                nc.vector.reciprocal(rv, mx)
                r127 = work.tile([P, 1], F32, tag="r127")
                nc.vector.tensor_scalar(
                    out=r127, in0=rv, scalar1=127.0, scalar2=None, op0=OP.mult
                )
                # q = convert_to_int8(y * r127); HW converter rounds to
                # nearest (CoreSim truncates - hardware is truth here).
                t = work.tile([P, D], F32, tag="t")
                nc.vector.tensor_tensor(
                    out=t, in0=s1f, in1=_bcast_inner(r127, 1, D), op=OP.mult
                )
                nc.vector.tensor_copy(out=yq_sb[:, j, :], in_=t)
                # ship the multiplier actually used for quantization so the
                # host can divide by it exactly (HW reciprocal is approximate;
                # q/r127 cancels that error, q*amax/127 does not)
                nc.vector.tensor_copy(out=sy_sb[:, j : j + 1], in_=r127)

            nc.sync.dma_start(out=yq_v[base], in_=yq_sb)
            nc.sync.dma_start(out=sy_v[base], in_=sy_sb)

        if outer == 1:
            body(0)
        else:
            with tc.For_i(0, outer, 1) as it:
                body(it)

    nc.compile()
    return nc


# ---------------------------------------------------------------------------
# Host-side prep/finish + cached PJRT runner with per-core pipelining
# ---------------------------------------------------------------------------

_RUNNER = None


def _get_runner():
    global _RUNNER
    if _RUNNER is None:
        _RUNNER = _Runner()
    return _RUNNER


class _Runner:
    def __init__(self):
        import jax
        import jax.numpy as jnp
        from jax.sharding import Mesh, NamedSharding, PartitionSpec
        from jax.experimental.shard_map import shard_map
        from concourse import bass2jax

        self.jax = jax
        self.jnp = jnp
        bass2jax.install_neuronx_cc_hook()

        nc = build_nc()
        self.nc = nc

        partition_name = (
            nc.partition_id_tensor.name if nc.partition_id_tensor else None
        )
        in_names = []
        out_names = []
        out_avals = []
        for alloc in nc.m.functions[0].allocations:
            if not isinstance(alloc, mybir.MemoryLocationSet):
                continue
            name = alloc.memorylocations[0].name
            if alloc.kind == "ExternalInput":
                if name != partition_name:
                    in_names.append(name)
            elif alloc.kind == "ExternalOutput":
                shape = tuple(alloc.tensor_shape)
                dtype = mybir.dt.np(alloc.dtype)
                out_names.append(name)
                out_avals.append(jax.core.ShapedArray(shape, dtype))
        self.in_names = list(in_names)
        self.out_names = list(out_names)
        n_params = len(in_names)
        n_outs = len(out_avals)
        all_names = list(in_names) + list(out_names)
        if partition_name is not None:
            all_names.append(partition_name)

        self.devices = jax.devices()[:N_CORES]
        assert len(self.devices) == N_CORES
        self.mesh = Mesh(np.asarray(self.devices), ("core",))
        self.sharding = NamedSharding(self.mesh, PartitionSpec("core"))
        donate = tuple(range(n_params, n_params + n_outs))

        def _body(*args):
            operands = list(args)
            if partition_name is not None:
                operands.append(bass2jax.partition_id_tensor())
            outs = bass2jax._bass_exec_p.bind(
                *operands,
                out_avals=tuple(out_avals),
                in_names=tuple(all_names),
                out_names=tuple(out_names),
                lowering_input_output_aliases=(),
                sim_require_finite=True,
                sim_require_nnan=True,
                nc=nc,
            )
            return tuple(outs)

        in_specs = (PartitionSpec("core"),) * (n_params + n_outs)
        out_specs = (PartitionSpec("core"),) * n_outs
        self._exec = jax.jit(
            shard_map(
                _body,
                mesh=self.mesh,
                in_specs=in_specs,
                out_specs=out_specs,
                check_rep=False,
            ),
            donate_argnums=donate,
            keep_unused=True,
        )

        sh = self.sharding
        self._zeros = jax.jit(
            lambda: jnp.zeros((N_TOKENS, OUT_ROW_BYTES), jnp.uint8),
            out_shardings=sh,
        )

        self.cpu = jax.devices("cpu")[0]

        def _prep_pair(x, gate_W, gate_b):
            # x: [2*SHARD, D] two cores' tokens -> blobs [2*ROWS_IN, 128]
            logits = x @ gate_W.T + gate_b
            m = jnp.max(logits, axis=-1, keepdims=True)
            eg = jnp.exp(logits - m)
            g = eg / jnp.sum(eg, axis=-1, keepdims=True)
            _, top2 = jax.lax.top_k(g, 2)
            iota = jnp.arange(E, dtype=top2.dtype)[None, :]
            mask = (iota == top2[:, 0:1]) | (iota == top2[:, 1:2])
            gm = jnp.where(mask, g, 0.0)
            amax = jnp.maximum(jnp.max(jnp.abs(x), axis=1), 1e-20)
            xq = jnp.rint(x * (127.0 / amax)[:, None]).astype(jnp.int8)
            gms = (gm * (amax / 127.0)[:, None]).astype(jnp.float16)
            gmp = (
                gms.reshape(2, NGROUP, G, P, E)
                .transpose(0, 1, 3, 2, 4)
                .reshape(2, NGROUP * P, G * E)
            )
            gt = jnp.rint(
                g.reshape(2, SHARD, E).transpose(0, 2, 1) * 255.0
            ).astype(jnp.uint8)
            blob = jnp.concatenate(
                [
                    jax.lax.bitcast_convert_type(xq, jnp.uint8).reshape(
                        2, ROWS_XQ, 128
                    ),
                    jax.lax.bitcast_convert_type(gmp, jnp.uint8).reshape(
                        2, ROWS_GMP, 128
                    ),
                    gt.reshape(2, ROWS_GT, 128),
                ],
                axis=1,
            )
            return blob

        self._prep_pair = jax.jit(_prep_pair, device=self.cpu)

        self._const_key = None
        self._const_dev = {}

    def _ensure_consts(self, gate_W, gate_b, W, b):
        key = (
            float(np.sum(W)),
            float(np.sum(b)),
            float(np.sum(gate_W)),
            float(np.sum(gate_b)),
        )
        if self._const_key == key:
            return
        jax = self.jax
        wcat = np.ascontiguousarray(
            W.transpose(2, 0, 1).reshape(D, E * D).astype(np.float32)
        )
        bmat = (b / 255.0).astype(np.float16)
        ident = np.eye(P, dtype=np.float32)
        consts = {
            "wcat": np.concatenate([wcat] * N_CORES, axis=0),
            "bmat": np.concatenate([bmat] * N_CORES, axis=0),
            "ident_f": np.concatenate([ident] * N_CORES, axis=0),
        }
        dbg = self.nc.dbg_addr
        if dbg is not None:
            consts[dbg.name] = np.zeros((N_CORES, 2), np.uint32)
        self._const_dev = {
            k: jax.device_put(v, self.sharding) for k, v in consts.items()
        }
        self._const_key = key

    def run(self, x, gate_W, gate_b, W, b):
        import threading
        import queue

        jax = self.jax
        self._ensure_consts(gate_W, gate_b, W, b)
        gw = gate_W.astype(np.float32)
        gbias = gate_b.astype(np.float32)

        put_q = queue.Queue()
        bufs = [None] * N_CORES
        put_err = []

        def put_worker():
            try:
                for _ in range(N_CORES):
                    c, blob_np = put_q.get()
                    bufs[c] = jax.device_put(blob_np, self.devices[c])
            except Exception as e:  # surfaced after join
                put_err.append(e)

        th = threading.Thread(target=put_worker)
        th.start()
        z_obuf = self._zeros()  # device-side, no wire; dispatch early
        for pair in range(N_CORES // 2):
            with jax.default_device(self.cpu):
                blob2 = self._prep_pair(
                    x[pair * 2 * SHARD : (pair + 1) * 2 * SHARD], gw, gbias
                )
            b2 = np.asarray(blob2)
            put_q.put((2 * pair, b2[0]))
            put_q.put((2 * pair + 1, b2[1]))
        th.join()
        if put_err:
            raise put_err[0]

        blob_glob = jax.make_array_from_single_device_arrays(
            (N_CORES * ROWS_IN, 128), self.sharding, bufs
        )
        args = []
        for name in self.in_names:
            if name == "blob":
                args.append(blob_glob)
            else:
                args.append(self._const_dev[name])
        (obuf_arr,) = self._exec(*args, z_obuf)

        out = np.empty((N_TOKENS, D), np.float32)
        shards = sorted(
            obuf_arr.addressable_shards, key=lambda s: s.index[0].start or 0
        )
        fetch_q = queue.Queue()
        fetch_err = []

        def fetch_one(s):
            try:
                fetch_q.put(((s.index[0].start or 0), np.asarray(s.data)))
            except Exception as e:
                fetch_err.append(e)
                fetch_q.put(None)

        fetchers = [
            threading.Thread(target=fetch_one, args=(s,)) for s in shards
        ]
        for t in fetchers:
            t.start()
        for _ in range(N_CORES):
            item = fetch_q.get()
            if item is None:
                break
            tok0, ob = item
            q = ob[:, 0:D].view(np.int8)
            syv = (
                np.ascontiguousarray(ob[:, D : D + 2])
                .view(np.float16)
                .astype(np.float32)
            )
            blockf = q.astype(np.float32)
            blockf *= 1.0 / syv
            out[tok0 : tok0 + ob.shape[0]] = blockf
        for t in fetchers:
            t.join()
        if fetch_err:
            raise fetch_err[0]
        return out


def kernel(**inputs) -> np.ndarray:
    global _RUNNER
    x = np.ascontiguousarray(np.asarray(inputs["x"], dtype=np.float32))
    gate_W = np.asarray(inputs["gate_W"], dtype=np.float32)
    gate_b = np.asarray(inputs["gate_b"], dtype=np.float32)
    W = np.asarray(inputs["W"], dtype=np.float32)
    b = np.asarray(inputs["b"], dtype=np.float32)
    try:
        return _get_runner().run(x, gate_W, gate_b, W, b)
    except Exception:
        # the axon tunnel occasionally drops a worker mid-call; rebuild the
        # runner (compile caches stay warm) and retry once
        _RUNNER = None
        import time as _time

        _time.sleep(5)
        return _get_runner().run(x, gate_W, gate_b, W, b)


# revision 22
# speedup vs baseline: 1.0357x; 1.0357x over previous
"""MoE top-2 routing kernel for Trainium2, 8-core data-parallel, int8 wire.

Problem: x [524288, 128] f32; gate Linear(128->8); 8 experts Linear(128->128).
  g = softmax(x @ gate_W.T + gate_b); top-2 mask; out = sum_e (g*mask)_e * (x @ W_e.T) + g @ b

The axon tunnel to the 8 NeuronCores moves ~40-48 MB/s half-duplex, so wall
time is dominated by wire bytes.  Strategy:
  host (jax CPU jit): gating/softmax/top-2 in f32 (exact), per-token int8
    quantization of x, dequant scale folded into the gate weights; final
    dequant of the device's int8 output.
  wire in:  one blob per core = xq int8 + gm*scale fp16 + gT fp16 (10MB/core)
  device:   int8->f32r convert, PE transpose, f32r expert matmuls, weighted
    bf16 reduce, g@b bias matmul, per-token amax + int8 quantization (HW
    converter rounds to nearest).
  wire out: one blob per core = yq int8 + r127 scale f32 (132B/token)

The PJRT exec path mirrors concourse.bass2jax.run_bass_via_pjrt but caches
the jitted sharded callable and keeps constant weights device-resident.
Host prep / transfers / fetch / dequant are pipelined per core.
"""

import sys

if "/opt/trn_rl_repo" not in sys.path:
    sys.path.insert(0, "/opt/trn_rl_repo")

from contextlib import ExitStack

import numpy as np

import concourse.bass as bass
import concourse.tile as tile
from concourse import bacc
from concourse import mybir

F32 = mybir.dt.float32
F32R = mybir.dt.float32r
BF16 = mybir.dt.bfloat16
F16 = mybir.dt.float16
I8 = mybir.dt.int8
U8 = mybir.dt.uint8
AF = mybir.ActivationFunctionType
OP = mybir.AluOpType
AX = mybir.AxisListType

N_TOKENS = 524288
D = 128
E = 8
N_CORES = 8
P = 128
G = 16  # tiles per group
SHARD = N_TOKENS // N_CORES  # 65536 tokens per core
NGROUP = SHARD // (P * G)  # 32 groups per core

# blob row layout (128-byte rows, uint8)
ROWS_XQ = SHARD  # int8 x, one row per token
ROWS_GV = SHARD * 2 // 128  # fp16 top-k gate value * scale, one slot block
ROWS_CD = SHARD // 128  # uint8 top-k expert index, one slot block
ROWS_GT = SHARD * E // 128  # uint8 g*255 transposed [E, SHARD]
ROWS_IN = ROWS_XQ + 2 * ROWS_GV + 2 * ROWS_CD + ROWS_GT
OUT_ROW_BYTES = D + 2  # int8 y row + fp16 r127 scale


def _bcast_inner(ap, n_outer, rep_len):
    """View [P, n_outer] as [P, n_outer, rep_len] with inner dim broadcast."""
    return bass.AP(
        tensor=ap.tensor,
        offset=ap.offset,
        ap=[ap.ap[0], [ap.ap[-1][0], n_outer], [0, rep_len]],
    )


def build_nc(shard_tokens: int = SHARD, inner_tiles: int = G) -> bass.Bass:
    ntiles = shard_tokens // P
    assert ntiles % inner_tiles == 0
    outer = ntiles // inner_tiles
    gi = inner_tiles
    rows_xq = shard_tokens
    rows_gv = shard_tokens * 2 // 128
    rows_cd = shard_tokens // 128
    rows_gt = shard_tokens * E // 128

    nc = bacc.Bacc()
    blob = nc.dram_tensor(
        "blob",
        [rows_xq + 2 * rows_gv + 2 * rows_cd + rows_gt, 128],
        U8,
        kind="ExternalInput",
    )
    # wcat[d, e*128+f] = W[e, f, d]
    wcat = nc.dram_tensor("wcat", [D, E * D], F32R, kind="ExternalInput")
    bmat = nc.dram_tensor("bmat", [E, D], F16, kind="ExternalInput")
    # iotaf[p, a*E + e] = e  (f32 compare target for index decode)
    iotaf = nc.dram_tensor("iotaf", [P, inner_tiles * E], F32, kind="ExternalInput")
    ident_f = nc.dram_tensor("ident_f", [P, P], F32R, kind="ExternalInput")
    obuf = nc.dram_tensor(
        "obuf", [shard_tokens, OUT_ROW_BYTES], U8, kind="ExternalOutput"
    )

    # views into the blobs
    x_v = (
        blob[0:rows_xq, :]
        .bitcast(I8)
        .rearrange("(n a p) d -> n p a d", p=P, a=gi)
    )
    # per-group [p, a] views of the de-interleaved top-2 value/index blocks
    def _slot_view(row0, nrows, dt):
        v = (
            blob[row0 : row0 + nrows, :]
            .rearrange("(n r) c -> n (r c)", n=outer)
            .rearrange("n (p h) -> n p h", p=P)
        )
        return v if dt is U8 else v.bitcast(dt)

    gv1_v = _slot_view(rows_xq, rows_gv, F16)
    gv2_v = _slot_view(rows_xq + rows_gv, rows_gv, F16)
    c1_v = _slot_view(rows_xq + 2 * rows_gv, rows_cd, U8)
    c2_v = _slot_view(rows_xq + 2 * rows_gv + rows_cd, rows_cd, U8)
    # gt logical [E, shard] uint8 (g * 255)
    gt_v = (
        blob[rows_xq + 2 * rows_gv + 2 * rows_cd :, :]
        .rearrange("(e r) c -> e (r c)", e=E)
        .rearrange("e (n t) -> n e t", t=P * gi)
    )
    yq_v = obuf[:, 0:D].bitcast(I8).rearrange("(n a p) d -> n p a d", p=P, a=gi)
    sy_v = (
        obuf[:, D : D + 2]
        .bitcast(F16)
        .rearrange("(n a p) one -> n p (a one)", p=P, a=gi)
    )

    with ExitStack() as ctx:
        tc = ctx.enter_context(tile.TileContext(nc))
        consts = ctx.enter_context(tc.tile_pool(name="consts", bufs=1))
        io_pool = ctx.enter_context(tc.tile_pool(name="io", bufs=2))
        xt_pool = ctx.enter_context(tc.tile_pool(name="xts", bufs=2))
        work = ctx.enter_context(tc.tile_pool(name="work", bufs=2))
        gates = ctx.enter_context(tc.tile_pool(name="gates", bufs=2))
        psum_y = ctx.enter_context(tc.tile_pool(name="psum_y", bufs=2, space="PSUM"))
        psum_t = ctx.enter_context(tc.tile_pool(name="psum_t", bufs=2, space="PSUM"))

        # ---- constants (one-time) ----
        wcat_sb = consts.tile([D, E * D], F32R)
        nc.sync.dma_start(out=wcat_sb, in_=wcat[:, :])
        bmat_sb = consts.tile([E, D], F16)
        nc.sync.dma_start(out=bmat_sb, in_=bmat[:, :])
        iota_sb = consts.tile([P, gi * E], F32)
        nc.sync.dma_start(out=iota_sb, in_=iotaf[:, :])
        ident_r = consts.tile([P, P], F32R)
        nc.sync.dma_start(out=ident_r, in_=ident_f[:, :])

        def body(base):
            xq_in = io_pool.tile([P, gi, D], I8, tag="xq_in")
            nc.sync.dma_start(out=xq_in, in_=x_v[base])
            gv1_in = gates.tile([P, gi], F16, tag="gv1_in")
            nc.sync.dma_start(out=gv1_in, in_=gv1_v[base])
            gv2_in = gates.tile([P, gi], F16, tag="gv2_in")
            nc.sync.dma_start(out=gv2_in, in_=gv2_v[base])
            c1_in = gates.tile([P, gi], U8, tag="c1_in")
            nc.sync.dma_start(out=c1_in, in_=c1_v[base])
            c2_in = gates.tile([P, gi], U8, tag="c2_in")
            nc.sync.dma_start(out=c2_in, in_=c2_v[base])
            # decode: gm[p, a, e] = v1*(e==c1) + v2*(e==c2), all contiguous
            # f32 operands + baseline-class stride-1 inner broadcasts
            v1f = gates.tile([P, gi], F32, tag="v1f")
            nc.vector.tensor_copy(out=v1f, in_=gv1_in)
            v2f = gates.tile([P, gi], F32, tag="v2f")
            nc.vector.tensor_copy(out=v2f, in_=gv2_in)
            c1f = gates.tile([P, gi], F32, tag="c1f")
            nc.vector.tensor_copy(out=c1f, in_=c1_in)
            c2f = gates.tile([P, gi], F32, tag="c2f")
            nc.vector.tensor_copy(out=c2f, in_=c2_in)
            io3 = iota_sb.rearrange("p (a e) -> p a e", e=E)
            eq1 = gates.tile([P, gi, E], F32, tag="eq1")
            nc.vector.tensor_tensor(
                out=eq1, in0=io3, in1=_bcast_inner(c1f, gi, E), op=OP.is_equal
            )
            eq2 = gates.tile([P, gi, E], F32, tag="eq2")
            nc.vector.tensor_tensor(
                out=eq2, in0=io3, in1=_bcast_inner(c2f, gi, E), op=OP.is_equal
            )
            t1 = gates.tile([P, gi, E], F32, tag="t1")
            nc.vector.tensor_tensor(
                out=t1, in0=eq1, in1=_bcast_inner(v1f, gi, E), op=OP.mult
            )
            t2 = gates.tile([P, gi, E], F32, tag="t2")
            nc.vector.tensor_tensor(
                out=t2, in0=eq2, in1=_bcast_inner(v2f, gi, E), op=OP.mult
            )
            gmf = gates.tile([P, gi * E], F32, tag="gmf")
            nc.vector.tensor_tensor(
                out=gmf.rearrange("p (a e) -> p a e", e=E),
                in0=t1,
                in1=t2,
                op=OP.add,
            )
            gt_u8 = gates.tile([E, gi * P], U8, tag="gt_u8")
            nc.sync.dma_start(out=gt_u8, in_=gt_v[base])
            gt_sb = gates.tile([E, gi * P], F16, tag="gt_sb")
            nc.vector.tensor_copy(out=gt_sb, in_=gt_u8)

            xts = xt_pool.tile([P, gi, D], F32R, tag="xts")
            yq_sb = io_pool.tile([P, gi, D], I8, tag="yq_sb")
            sy_sb = io_pool.tile([P, gi], F16, tag="sy_sb")

            for j in range(gi):
                # int8 -> f32r (exact, ACT convert), then PE transpose -> xT
                xf = work.tile([P, D], F32R, tag="xf")
                nc.scalar.activation(xf, xq_in[:, j, :], AF.Copy)
                tp = psum_t.tile([P, D], F32, tag="tp")
                nc.tensor.transpose(tp.bitcast(F32R), xf, ident_r)
                nc.scalar.copy(xts[:, j, :], tp)

                yp = psum_y.tile([P, E * D], F32, tag="yall")
                nc.tensor.matmul(
                    yp[:, 0:512], xts[:, j, :], wcat_sb[:, 0:512], start=True, stop=True
                )
                nc.tensor.matmul(
                    yp[:, 512:1024],
                    xts[:, j, :],
                    wcat_sb[:, 512:1024],
                    start=True,
                    stop=True,
                )

                # weighted reduce: sc[p, e, f] = yall[p, e, f] * gm_scaled[p, j, e]
                sc = work.tile([P, E, D], BF16, tag="sc")
                yp3 = yp.rearrange("p (e f) -> p e f", f=D)
                ghj = gmf[:, j * E : (j + 1) * E]
                ghb = bass.AP(
                    tensor=ghj.tensor,
                    offset=ghj.offset,
                    ap=[ghj.ap[0], [1, 6], [0, D]],
                )
                nc.vector.tensor_tensor(
                    out=sc[:, 0:6, :], in0=yp3[:, 0:6, :], in1=ghb, op=OP.mult
                )
                for e in (6, 7):
                    nc.scalar.activation(
                        sc[:, e, :], yp3[:, e, :], AF.Copy, scale=ghj[:, e : e + 1]
                    )
                # bf16 add tree: level 1 on gpsimd, 2 on DVE, final f32 on DVE
                sc4 = work.tile([P, 4, D], BF16, tag="sc4")
                nc.gpsimd.tensor_tensor(
                    out=sc4, in0=sc[:, 0:4, :], in1=sc[:, 4:8, :], op=OP.add
                )
                sc2 = work.tile([P, 2, D], BF16, tag="sc2")
                nc.vector.tensor_tensor(
                    out=sc2, in0=sc4[:, 0:2, :], in1=sc4[:, 2:4, :], op=OP.add
                )
                # bias term: bp[p, f] = sum_e g[p, e] * b[e, f]
                bp = psum_t.tile([P, D], F32, tag="bp")
                nc.tensor.matmul(
                    bp,
                    gt_sb[:, j * P : (j + 1) * P],
                    bmat_sb,
                    start=True,
                    stop=True,
                )
                s0 = work.tile([P, D], F32, tag="s0")
                nc.vector.tensor_tensor(
                    out=s0, in0=sc2[:, 0, :], in1=sc2[:, 1, :], op=OP.add
                )
                s1f = work.tile([P, D], F32, tag="s1f")
                nc.vector.tensor_tensor(out=s1f, in0=s0, in1=bp, op=OP.add)

                # per-token int8 quantization: q = rne(y * 127/amax)
                ab = work.tile([P, D], F32, tag="ab")
                nc.scalar.activation(ab, s1f, AF.Abs)
                mx = work.tile([P, 1], F32, tag="mx")
                nc.vector.tensor_reduce(out=mx, in_=ab, axis=AX.X, op=OP.max)
                nc.vector.tensor_scalar(
                    out=mx, in0=mx, scalar1=1e-30, scalar2=None, op0=OP.max
                )
                rv = work.tile([P, 1], F32, tag="rv")
                nc.vector.reciprocal(rv, mx)
                r127 = work.tile([P, 1], F32, tag="r127")
                nc.vector.tensor_scalar(
                    out=r127, in0=rv, scalar1=127.0, scalar2=None, op0=OP.mult
                )
                # q = convert_to_int8(y * r127); HW converter rounds to
                # nearest (CoreSim truncates - hardware is truth here).
                t = work.tile([P, D], F32, tag="t")
                nc.vector.tensor_tensor(
                    out=t, in0=s1f, in1=_bcast_inner(r127, 1, D), op=OP.mult
                )
                nc.vector.tensor_copy(out=yq_sb[:, j, :], in_=t)
                # ship the multiplier actually used for quantization so the
                # host can divide by it exactly (HW reciprocal is approximate;
                # q/r127 cancels that error, q*amax/127 does not)
                nc.vector.tensor_copy(out=sy_sb[:, j : j + 1], in_=r127)

            nc.sync.dma_start(out=yq_v[base], in_=yq_sb)
            nc.sync.dma_start(out=sy_v[base], in_=sy_sb)

        if outer == 1:
            body(0)
        else:
            with tc.For_i(0, outer, 1) as it:
                body(it)

    nc.compile()
    return nc


# ---------------------------------------------------------------------------
# Host-side prep/finish + cached PJRT runner with per-core pipelining
# ---------------------------------------------------------------------------

_RUNNER = None


def _get_runner():
    global _RUNNER
    if _RUNNER is None:
        _RUNNER = _Runner()
    return _RUNNER


class _Runner:
    def __init__(self):
        import jax
        import jax.numpy as jnp
        from jax.sharding import Mesh, NamedSharding, PartitionSpec
        from jax.experimental.shard_map import shard_map
        from concourse import bass2jax

        self.jax = jax
        self.jnp = jnp
        bass2jax.install_neuronx_cc_hook()

        nc = build_nc()
        self.nc = nc

        partition_name = (
            nc.partition_id_tensor.name if nc.partition_id_tensor else None
        )
        in_names = []
        out_names = []
        out_avals = []
        for alloc in nc.m.functions[0].allocations:
            if not isinstance(alloc, mybir.MemoryLocationSet):
                continue
            name = alloc.memorylocations[0].name
            if alloc.kind == "ExternalInput":
                if name != partition_name:
                    in_names.append(name)
            elif alloc.kind == "ExternalOutput":
                shape = tuple(alloc.tensor_shape)
                dtype = mybir.dt.np(alloc.dtype)
                out_names.append(name)
                out_avals.append(jax.core.ShapedArray(shape, dtype))
        self.in_names = list(in_names)
        self.out_names = list(out_names)
        n_params = len(in_names)
        n_outs = len(out_avals)
        all_names = list(in_names) + list(out_names)
        if partition_name is not None:
            all_names.append(partition_name)

        self.devices = jax.devices()[:N_CORES]
        assert len(self.devices) == N_CORES
        self.mesh = Mesh(np.asarray(self.devices), ("core",))
        self.sharding = NamedSharding(self.mesh, PartitionSpec("core"))
        donate = tuple(range(n_params, n_params + n_outs))

        def _body(*args):
            operands = list(args)
            if partition_name is not None:
                operands.append(bass2jax.partition_id_tensor())
            outs = bass2jax._bass_exec_p.bind(
                *operands,
                out_avals=tuple(out_avals),
                in_names=tuple(all_names),
                out_names=tuple(out_names),
                lowering_input_output_aliases=(),
                sim_require_finite=True,
                sim_require_nnan=True,
                nc=nc,
            )
            return tuple(outs)

        in_specs = (PartitionSpec("core"),) * (n_params + n_outs)
        out_specs = (PartitionSpec("core"),) * n_outs
        self._exec = jax.jit(
            shard_map(
                _body,
                mesh=self.mesh,
                in_specs=in_specs,
                out_specs=out_specs,
                check_rep=False,
            ),
            donate_argnums=donate,
            keep_unused=True,
        )

        sh = self.sharding
        self._zeros = jax.jit(
            lambda: jnp.zeros((N_TOKENS, OUT_ROW_BYTES), jnp.uint8),
            out_shardings=sh,
        )

        self.cpu = jax.devices("cpu")[0]

        def _prep_pair(x, gate_W, gate_b):
            # x: [2*SHARD, D] two cores' tokens -> blobs [2*ROWS_IN, 128]
            logits = x @ gate_W.T + gate_b
            m = jnp.max(logits, axis=-1, keepdims=True)
            eg = jnp.exp(logits - m)
            g = eg / jnp.sum(eg, axis=-1, keepdims=True)
            _, top2 = jax.lax.top_k(g, 2)
            amax = jnp.maximum(jnp.max(jnp.abs(x), axis=1), 1e-20)
            xq = jnp.rint(x * (127.0 / amax)[:, None]).astype(jnp.int8)
            vals = (
                jnp.take_along_axis(g, top2, axis=1)
                * (amax / 127.0)[:, None]
            ).astype(jnp.float16)

            def pack_slot(arr):
                # [2*SHARD] per-token -> per-core [NGROUP*P, G] group layout
                return (
                    arr.reshape(2, NGROUP, G, P)
                    .transpose(0, 1, 3, 2)
                    .reshape(2, NGROUP * P, G)
                )

            gv1 = pack_slot(vals[:, 0])
            gv2 = pack_slot(vals[:, 1])
            cd1 = pack_slot(top2[:, 0].astype(jnp.uint8))
            cd2 = pack_slot(top2[:, 1].astype(jnp.uint8))
            gt = jnp.rint(
                g.reshape(2, SHARD, E).transpose(0, 2, 1) * 255.0
            ).astype(jnp.uint8)
            blob = jnp.concatenate(
                [
                    jax.lax.bitcast_convert_type(xq, jnp.uint8).reshape(
                        2, ROWS_XQ, 128
                    ),
                    jax.lax.bitcast_convert_type(gv1, jnp.uint8).reshape(
                        2, ROWS_GV, 128
                    ),
                    jax.lax.bitcast_convert_type(gv2, jnp.uint8).reshape(
                        2, ROWS_GV, 128
                    ),
                    cd1.reshape(2, ROWS_CD, 128),
                    cd2.reshape(2, ROWS_CD, 128),
                    gt.reshape(2, ROWS_GT, 128),
                ],
                axis=1,
            )
            return blob

        self._prep_pair = jax.jit(_prep_pair, device=self.cpu)

        self._const_key = None
        self._const_dev = {}

    def _ensure_consts(self, gate_W, gate_b, W, b):
        key = (
            float(np.sum(W)),
            float(np.sum(b)),
            float(np.sum(gate_W)),
            float(np.sum(gate_b)),
        )
        if self._const_key == key:
            return
        jax = self.jax
        wcat = np.ascontiguousarray(
            W.transpose(2, 0, 1).reshape(D, E * D).astype(np.float32)
        )
        bmat = (b / 255.0).astype(np.float16)
        iotaf = np.tile(
            np.tile(np.arange(E, dtype=np.float32), G), (P, 1)
        )
        ident = np.eye(P, dtype=np.float32)
        consts = {
            "wcat": np.concatenate([wcat] * N_CORES, axis=0),
            "bmat": np.concatenate([bmat] * N_CORES, axis=0),
            "iotaf": np.concatenate([iotaf] * N_CORES, axis=0),
            "ident_f": np.concatenate([ident] * N_CORES, axis=0),
        }
        dbg = self.nc.dbg_addr
        if dbg is not None:
            consts[dbg.name] = np.zeros((N_CORES, 2), np.uint32)
        self._const_dev = {
            k: jax.device_put(v, self.sharding) for k, v in consts.items()
        }
        self._const_key = key

    def run(self, x, gate_W, gate_b, W, b):
        import threading
        import queue

        jax = self.jax
        self._ensure_consts(gate_W, gate_b, W, b)
        gw = gate_W.astype(np.float32)
        gbias = gate_b.astype(np.float32)

        put_q = queue.Queue()
        bufs = [None] * N_CORES
        put_err = []

        def put_worker():
            try:
                for _ in range(N_CORES):
                    c, blob_np = put_q.get()
                    bufs[c] = jax.device_put(blob_np, self.devices[c])
            except Exception as e:  # surfaced after join
                put_err.append(e)

        th = threading.Thread(target=put_worker)
        th.start()
        z_obuf = self._zeros()  # device-side, no wire; dispatch early
        for pair in range(N_CORES // 2):
            with jax.default_device(self.cpu):
                blob2 = self._prep_pair(
                    x[pair * 2 * SHARD : (pair + 1) * 2 * SHARD], gw, gbias
                )
            b2 = np.asarray(blob2)
            put_q.put((2 * pair, b2[0]))
            put_q.put((2 * pair + 1, b2[1]))
        th.join()
        if put_err:
            raise put_err[0]

        blob_glob = jax.make_array_from_single_device_arrays(
            (N_CORES * ROWS_IN, 128), self.sharding, bufs
        )
        args = []
        for name in self.in_names:
            if name == "blob":
                args.append(blob_glob)
            else:
                args.append(self._const_dev[name])
        (obuf_arr,) = self._exec(*args, z_obuf)

        out = np.empty((N_TOKENS, D), np.float32)
        shards = sorted(
            obuf_arr.addressable_shards, key=lambda s: s.index[0].start or 0
        )
        fetch_q = queue.Queue()
        fetch_err = []

        def fetch_one(s):
            try:
                fetch_q.put(((s.index[0].start or 0), np.asarray(s.data)))
            except Exception as e:
                fetch_err.append(e)
                fetch_q.put(None)

        fetchers = [
            threading.Thread(target=fetch_one, args=(s,)) for s in shards
        ]
        for t in fetchers:
            t.start()
        for _ in range(N_CORES):
            item = fetch_q.get()
            if item is None:
                break
            tok0, ob = item
            q = ob[:, 0:D].view(np.int8)
            syv = (
                np.ascontiguousarray(ob[:, D : D + 2])
                .view(np.float16)
                .astype(np.float32)
            )
            blockf = q.astype(np.float32)
            blockf *= 1.0 / syv
            out[tok0 : tok0 + ob.shape[0]] = blockf
        for t in fetchers:
            t.join()
        if fetch_err:
            raise fetch_err[0]
        return out


def kernel(**inputs) -> np.ndarray:
    global _RUNNER
    x = np.ascontiguousarray(np.asarray(inputs["x"], dtype=np.float32))
    gate_W = np.asarray(inputs["gate_W"], dtype=np.float32)
    gate_b = np.asarray(inputs["gate_b"], dtype=np.float32)
    W = np.asarray(inputs["W"], dtype=np.float32)
    b = np.asarray(inputs["b"], dtype=np.float32)
    try:
        return _get_runner().run(x, gate_W, gate_b, W, b)
    except Exception:
        # the axon tunnel occasionally drops a worker mid-call; rebuild the
        # runner (compile caches stay warm) and retry once
        _RUNNER = None
        import time as _time

        _time.sleep(5)
        return _get_runner().run(x, gate_W, gate_b, W, b)
